# revision 9
# baseline (speedup 1.0000x reference)
"""MoE layer (top-k routing) on 8 Trainium2 NeuronCores.

Expert-parallel per the sharding hint: the host computes router softmax +
top-k (0.1% of FLOPs) and realizes the "all-to-all dispatch by expert
assignment" while building the per-core SPMD input maps; each core runs the
expert FFN in bf16 (fp32 PSUM accumulation) with the combine weight
multiplied in on-device; the host scatter-adds results back to [B,N,C].

For load balance, each expert's FFN is split along the D_FF axis into two
half-units (exact: gelu is elementwise over F,
and GEMM2 contracts F, so y = y_half0 + y_half1). The 16 half-units are
assigned two per core: the 4 largest experts' halves fill slot class A, the
4 smallest fill slot class B. The SPMD program pads slot A to the largest
top-4 count and slot B to the largest bottom-4 count, so per-core padded
work drops from 2*max(counts) to max(top4)+max(bottom4). Host sums the two
half partials per expert during the scatter-add.
"""

import json
import os
import sys
import types

import numpy as np
import ml_dtypes

D_MODEL = 1024
D_FF = 4096
N_EXPERTS = 8
N_CORES = 8

P = 128
CB = D_MODEL // P      # 8 c-blocks of 128
FH = D_FF // 2         # F half = 2048
FBH = FH // P          # 16 f-blocks per half
TN = 512               # token tile (matmul moving free dim / one PSUM bank)


def _shim_axon_hooks():
    if "antenv.axon_hooks" in sys.modules:
        return
    try:
        import trn_agent_boot.trn_boot as _tb
        hook = _tb._ntff_profile_via_ctypes("/opt/axon/libaxon_pjrt.so")
    except Exception:
        hook = None
    mod = types.ModuleType("antenv.axon_hooks")
    mod.get_axon_ntff_profile_hook = lambda: hook
    mod.set_axon_ntff_profile_hook = lambda h: None
    sys.modules["antenv.axon_hooks"] = mod


_shim_axon_hooks()

import concourse.bass as bass            # noqa: E402
import concourse.tile as tile            # noqa: E402
from concourse import mybir              # noqa: E402
from concourse.bass import ds, ts        # noqa: E402
from concourse.bass_utils import run_bass_kernel_spmd  # noqa: E402


def _fix_multiwait_bir(nc):
    """Split instructions carrying >1 sync wait (the TileContext tail drain)
    into single-wait NoOps; this walrus build rejects multi-wait CTRL
    instructions."""
    raw = bass.Bass.to_json_bytes(nc)
    d = json.loads(raw)
    for f in d["functions"]:
        for b in f["blocks"]:
            out = []
            for i in b["instructions"]:
                si = i.get("sync_info") or {}
                waits = si.get("on_wait") or []
                if len(waits) > 1:
                    for k, w in enumerate(waits[:-1]):
                        out.append({
                            "name": f"{i['name']}_wsplit{k}",
                            "engine": i["engine"],
                            "ins": [], "outs": [],
                            "opcode": "NoOp",
                            "sync_info": {"on_update": [], "on_wait": [w]},
                        })
                    si["on_wait"] = [waits[-1]]
                out.append(i)
            b["instructions"] = out
    fixed = json.dumps(d).encode()
    nc.to_json_bytes = lambda: fixed


_NC_CACHE = {}


def _token_tiles(cap, remainder_first=False):
    tiles, off = [], 0
    while off < cap:
        tw = min(TN, cap - off)
        tiles.append((off, tw))
        off += tw
    if remainder_first and len(tiles) > 1 and tiles[-1][1] < TN:
        # a small first tile shortens the PE-start critical path (its token
        # load is the first thing the first matmul waits on)
        tiles = [tiles[-1]] + tiles[:-1]
    return tiles


def _build_moe_kernel(cap_a, cap_b):
    """Two half-expert FFN units per core (slot A then slot B), SPMD x8."""
    key = (cap_a, cap_b)
    if key in _NC_CACHE:
        return _NC_CACHE[key]

    bf16 = mybir.dt.bfloat16
    f32 = mybir.dt.float32
    Act = mybir.ActivationFunctionType

    nc = bass.Bass("TRN2", target_bir_lowering=False, debug=False,
                   num_devices=N_CORES)

    units = []
    for slot, cap in (("A", cap_a), ("B", cap_b)):
        u = {"cap": cap, "slot": slot}
        u["xT"] = nc.declare_dram_parameter(f"xT{slot}", [D_MODEL, cap], bf16, isOutput=False)
        u["w1t"] = nc.declare_dram_parameter(f"w1t{slot}", [D_MODEL, FH], bf16, isOutput=False)
        u["w2t"] = nc.declare_dram_parameter(f"w2t{slot}", [FH, D_MODEL], bf16, isOutput=False)
        u["b1"] = nc.declare_dram_parameter(f"b1{slot}", [FH], f32, isOutput=False)
        u["b2"] = nc.declare_dram_parameter(f"b2{slot}", [D_MODEL], f32, isOutput=False)
        u["wts"] = nc.declare_dram_parameter(f"wts{slot}", [P, cap], f32, isOutput=False)
        u["yT"] = nc.declare_dram_parameter(f"yT{slot}", [D_MODEL, cap], f32, isOutput=True)
        u["xr"] = u["xT"].ap().rearrange("(g p) t -> p g t", p=P)
        u["w1r"] = u["w1t"].ap().rearrange("(g p) f -> p g f", p=P)   # [128, 8, 2048]
        u["w2r"] = u["w2t"].ap().rearrange("(g p) c -> p g c", p=P)   # [128, 16, 1024]
        u["b1r"] = u["b1"].ap().rearrange("(g p) -> p g", p=P)        # [128, 16]
        u["b2r"] = u["b2"].ap().rearrange("(g p) -> p g", p=P)        # [128, 8]
        u["yr"] = u["yT"].ap().rearrange("(g p) t -> p g t", p=P)
        u["tiles"] = _token_tiles(cap, remainder_first=(slot == "A"))
        units.append(u)

    MS = 512  # w1 M-strip width (~1 MiB per DMA)

    with tile.TileContext(nc) as tc:
        with (
            tc.tile_pool(name="weights", bufs=1) as wpool,
            tc.tile_pool(name="xin", bufs=3) as xpool,
            tc.tile_pool(name="wtp", bufs=3) as wtpool,
            tc.tile_pool(name="hbuf", bufs=1) as hpool,
            tc.tile_pool(name="yout", bufs=2) as ypool,
            tc.tile_pool(name="psum", bufs=4, space="PSUM") as psum,
        ):
            # ---- loads: unit A's critical path first, then the rest ----
            ua, ub = units
            a_off0, a_tw0 = ua["tiles"][0]
            ua["x0"] = xpool.tile([P, CB, TN], bf16, tag="xt", name="x0A")
            nc.sync.dma_start(ua["x0"][:, :, :a_tw0],
                              ua["xr"][:, :, ds(a_off0, a_tw0)])
            # first w1 strips ride the ACT HWDGE ring so they overlap x0A's
            # load on the SP ring — shortens the PE-start critical path;
            # a narrow leading strip gets the first matmul going sooner
            MS0 = 256
            ua["w1_sb"] = wpool.tile([P, CB, FH], bf16, tag="w1A", name="w1A")
            nc.scalar.dma_start(ua["w1_sb"][:, :, 0:MS0], ua["w1r"][:, :, 0:MS0])
            nc.scalar.dma_start(ua["w1_sb"][:, :, MS0:MS], ua["w1r"][:, :, MS0:MS])

            for u in units:
                u["b1_sb"] = wpool.tile([P, FBH], f32, tag=f"b1{u['slot']}", name=f"b1{u['slot']}")
                nc.sync.dma_start(u["b1_sb"][:], u["b1r"])
                u["b2_sb"] = wpool.tile([P, CB], f32, tag=f"b2{u['slot']}", name=f"b2{u['slot']}")
                nc.sync.dma_start(u["b2_sb"][:], u["b2r"])
            ua["wt0"] = wtpool.tile([P, TN], f32, tag="wt", name="wt0A")
            nc.sync.dma_start(ua["wt0"][:, :a_tw0], ua["wts"][:, ds(a_off0, a_tw0)])

            for s in range(MS, FH, MS):
                nc.sync.dma_start(ua["w1_sb"][:, :, s:s + MS], ua["w1r"][:, :, s:s + MS])
            ua["w2_sb"] = wpool.tile([P, FBH, D_MODEL], bf16, tag="w2A", name="w2A")
            for k in range(0, FBH, 4):
                nc.sync.dma_start(ua["w2_sb"][:, k:k + 4, :], ua["w2r"][:, k:k + 4, :])

            ub["w1_sb"] = wpool.tile([P, CB, FH], bf16, tag="w1B", name="w1B")
            for s in range(0, FH, MS):
                nc.sync.dma_start(ub["w1_sb"][:, :, s:s + MS], ub["w1r"][:, :, s:s + MS])
            ub["w2_sb"] = wpool.tile([P, FBH, D_MODEL], bf16, tag="w2B", name="w2B")
            for k in range(0, FBH, 4):
                nc.sync.dma_start(ub["w2_sb"][:, k:k + 4, :], ub["w2r"][:, k:k + 4, :])

            # ---- compute: unit A tiles, then unit B tiles ----
            for u in units:
                for ti, (off, tw) in enumerate(u["tiles"]):
                    if ti == 0 and "x0" in u:
                        xt, wt = u["x0"], u["wt0"]
                    else:
                        xt = xpool.tile([P, CB, TN], bf16, tag="xt")
                        nc.sync.dma_start(xt[:, :, :tw], u["xr"][:, :, ds(off, tw)])
                        wt = wtpool.tile([P, TN], f32, tag="wt")
                        nc.sync.dma_start(wt[:, :tw], u["wts"][:, ds(off, tw)])

                    ht = hpool.tile([P, FBH, TN], bf16, tag="ht")
                    for m in range(FBH):
                        ph = psum.tile([P, TN], f32, tag="ph")
                        for k in range(CB):
                            nc.tensor.matmul(
                                ph[:, :tw],
                                lhsT=u["w1_sb"][:, k, ts(m, P)],
                                rhs=xt[:, k, :tw],
                                start=(k == 0), stop=(k == CB - 1),
                            )
                        nc.scalar.activation(ht[:, m, :tw], ph[:, :tw], Act.Gelu,
                                             bias=u["b1_sb"][:, m:m + 1])
                    last = (u is ub) and (ti == len(u["tiles"]) - 1)
                    yt = ypool.tile([P, CB, TN], f32, tag="yt")
                    for c in range(CB):
                        py = psum.tile([P, TN], f32, tag="py")
                        for k in range(FBH):
                            nc.tensor.matmul(
                                py[:, :tw],
                                lhsT=u["w2_sb"][:, k, ts(c, P)],
                                rhs=ht[:, k, :tw],
                                start=(k == 0), stop=(k == FBH - 1),
                            )
                        nc.scalar.add(yt[:, c, :tw], py[:, :tw], u["b2_sb"][:, c:c + 1])
                        nc.vector.tensor_mul(yt[:, c, :tw], yt[:, c, :tw], wt[:, :tw])
                        if last:
                            # final tile: per-block stores overlap the tail
                            # GEMM2 blocks instead of one post-loop DMA
                            nc.sync.dma_start(u["yr"][:, c, ds(off, tw)],
                                              yt[:, c, :tw])
                    if not last:
                        nc.sync.dma_start(u["yr"][:, :, ds(off, tw)], yt[:, :, :tw])

    _fix_multiwait_bir(nc)
    _NC_CACHE[key] = nc
    return nc


def _route(xf, router_w, k):
    """Replicate the reference router numerics (f32 softmax, top-k, renorm)."""
    logits = xf @ router_w.T.astype(np.float32)          # [T, E]
    m = logits.max(axis=-1, keepdims=True)
    e = np.exp(logits - m, dtype=np.float32)
    probs = e / e.sum(axis=-1, keepdims=True)
    idx = np.argsort(-probs, axis=-1, kind="stable")[:, :k]   # [T, k]
    w = np.take_along_axis(probs, idx, axis=-1)               # [T, k]
    w = w / (w.sum(axis=-1, keepdims=True) + 1e-9)
    return idx, w


def _align16(n):
    return max(P, -(-n // 16) * 16)


def kernel(x, router_w, expert_w1, expert_b1, expert_w2, expert_b2, top_k):
    x = np.asarray(x)
    router_w = np.asarray(router_w, dtype=np.float32)
    expert_w1 = np.asarray(expert_w1, dtype=np.float32)
    expert_b1 = np.asarray(expert_b1, dtype=np.float32)
    expert_w2 = np.asarray(expert_w2, dtype=np.float32)
    expert_b2 = np.asarray(expert_b2, dtype=np.float32)
    k = int(np.asarray(top_k))
    Bq, Nq, C = x.shape
    Tq = Bq * Nq
    E = expert_w1.shape[0]
    xf = np.ascontiguousarray(x.reshape(Tq, C), dtype=np.float32)

    idx, w = _route(xf, router_w, k)

    tok_idx, tok_w = [], []
    for e in range(E):
        mask = idx == e
        sel = np.nonzero(mask.any(axis=-1))[0]
        tok_idx.append(sel)
        tok_w.append((w * mask).sum(axis=-1)[sel].astype(np.float32))
    counts = np.array([len(s) for s in tok_idx])

    # slot A <- both halves of the 4 largest experts; slot B <- 4 smallest.
    order = np.argsort(-counts, kind="stable")
    big, small = order[:4], order[4:]
    cap_a = _align16(int(counts[big].max()))
    cap_b = _align16(int(counts[small].max()) if len(small) else P)

    nc = _build_moe_kernel(cap_a, cap_b)

    def unit_inputs(e, half, cap, slot):
        cnt = counts[e]
        f0, f1 = half * FH, (half + 1) * FH
        xT = np.zeros((C, cap), dtype=ml_dtypes.bfloat16)
        xT[:, :cnt] = xf[tok_idx[e]].T
        wtsP = np.zeros((P, cap), dtype=np.float32)
        wtsP[:, :cnt] = tok_w[e][None, :]
        b2 = expert_b2[e] if half == 0 else np.zeros(C, dtype=np.float32)
        return {
            f"xT{slot}": xT,
            f"w1t{slot}": np.ascontiguousarray(expert_w1[e, f0:f1].T).astype(ml_dtypes.bfloat16),
            f"w2t{slot}": np.ascontiguousarray(expert_w2[e, :, f0:f1].T).astype(ml_dtypes.bfloat16),
            f"b1{slot}": np.ascontiguousarray(expert_b1[e, f0:f1]),
            f"b2{slot}": np.ascontiguousarray(b2),
            f"wts{slot}": wtsP,
        }

    # core 2i / 2i+1 take halves 0/1 of big[i] in slot A and of small[i] in B
    assign = []   # per core: ((eA, halfA), (eB, halfB))
    for i in range(4):
        for h in range(2):
            assign.append(((int(big[i]), h), (int(small[i]), h)))

    in_maps = []
    for (ea, ha), (eb, hb) in assign:
        m = unit_inputs(ea, ha, cap_a, "A")
        m.update(unit_inputs(eb, hb, cap_b, "B"))
        in_maps.append(m)

    trace = os.environ.get("BASS_MOE_TRACE") == "1"
    res = run_bass_kernel_spmd(
        nc, in_maps, core_ids=list(range(N_CORES)),
        trace=trace,
        tmpdir=os.environ.get("BASS_MOE_TMPDIR") if trace else None,
    )
    if trace:
        kernel.last_exec_time_ns = res.exec_time_ns
        kernel.last_trace = (res.instructions_and_trace or (None, None))[1]

    out = np.zeros((Tq, C), dtype=np.float32)
    for core, ((ea, _), (eb, _)) in enumerate(assign):
        if counts[ea]:
            out[tok_idx[ea]] += res.results[core]["yTA"][:, :counts[ea]].T
        if counts[eb]:
            out[tok_idx[eb]] += res.results[core]["yTB"][:, :counts[eb]].T
    return out.reshape(Bq, Nq, C).astype(x.dtype)
